# revision 1
# baseline (speedup 1.0000x reference)
"""AtomicConv radial symmetry function kernel for Trainium2 (8 NeuronCores).

Strategy:
  - Data-parallel over batch: 4 examples per core.
  - Host sorts each atom's 64 neighbors by atom-type into 4 slot ranges of
    16 (overflow beyond 16 per type is exactly corrected on host - it is
    statistically rare). Untyped neighbors are dropped (they contribute to
    no output). Padding slots point at a sentinel table row whose huge
    coords clamp to the cutoff radius, where the cosine window is 0.
  - Device per chunk of 1024 atoms:
      gpsimd.ap_gather fetches neighbor x/y/z (per-core index lists,
      components as channels) -> SBUF->SBUF DMA transposes into a
      slot-on-partition layout (p = half*64 + slot, free = atom) ->
      DVE computes clamped R^2, ACT computes R, the cosine window F and
      ln(F); the type mask and window are folded into the Gaussian via
      exp(-e(R-rs_l)^2 + ln F) -> PE contracts the 16-slot type ranges
      with a constant block-ones matrix, stacking all 16 radial filters
      in one PSUM tile -> one DVE drain + one DMA store per chunk.
"""

import numpy as np

B, N, M = 32, 2048, 64
L, A = 16, 4
NCORES = 8
BPC = B // NCORES          # examples per core
AT = BPC * N               # atoms per core (8192)
CH = 1024                  # atoms per chunk
NCHUNK = AT // CH          # 8
S = 64                     # slots per atom (16 per type)
PPT = 16                   # slots (padding) per type
HALF = CH // 2             # 512 atoms per half -> free dim
NIDX = CH * S // NCORES    # gathered idxs per gpsimd core per chunk (8192)
NE = AT + 16               # gather table entries (+ sentinel pad)
SENT = AT                  # sentinel table index
SENT_COORD = 1.0e4

_cache = {}


def _build_program(rc0, e0, rs_vals, ablate=""):
    import concourse.bacc as bacc
    import concourse.mybir as mybir
    from concourse.tile import TileContext
    from concourse import mybir as mb

    f32 = mybir.dt.float32
    i16 = mybir.dt.int16
    AF = mybir.ActivationFunctionType
    Alu = mybir.AluOpType

    nc = bacc.Bacc(None, target_bir_lowering=False)

    # register const APs for every activation bias value we use
    bias_vals = {float(np.pi / 2), 1.0e-38}
    for l in range(L):
        bias_vals.add(-float(e0) * float(rs_vals[l]) * float(rs_vals[l]))
    for v in sorted(bias_vals):
        if (f32, v) not in nc.const_aps.aps:
            t = nc.alloc_sbuf_tensor(f"cst-{v!r}", [128, 1], f32)
            nc.gpsimd.memset(t.ap(), v)
            nc.const_aps.aps[(f32, v)] = t.ap()
    nc.all_engine_barrier()

    tbl_d = nc.dram_tensor("tbl", (128, NE), f32, kind="ExternalInput")
    idx_d = nc.dram_tensor("idx", (128, NCHUNK * (NIDX // 16)), i16,
                           kind="ExternalInput")
    cen_d = nc.dram_tensor("cen", (128, 3 * NCHUNK * HALF), f32,
                           kind="ExternalInput")
    ones_d = nc.dram_tensor("ones", (128, 8), f32, kind="ExternalInput")
    out_d = nc.dram_tensor("out", (NCHUNK * 128, HALF), f32,
                           kind="ExternalOutput")

    ICOL = NIDX // 16     # 512 idx cols per chunk
    rc2 = float(rc0) * float(rc0)

    with TileContext(nc) as tc:
        with (
            tc.tile_pool(name="tab", bufs=1) as tab_pool,
            tc.tile_pool(name="gath", bufs=2) as gath_pool,
            tc.tile_pool(name="idxp", bufs=2) as idx_pool,
            tc.tile_pool(name="cenp", bufs=2) as cen_pool,
            tc.tile_pool(name="comp", bufs=2) as comp_pool,
            tc.tile_pool(name="ew", bufs=2) as ew_pool,
            tc.tile_pool(name="kt", bufs=3) as k_pool,
            tc.tile_pool(name="ot", bufs=2) as out_pool,
            tc.tile_pool(name="ps", bufs=2, space="PSUM") as psum_pool,
        ):
            t_tab = tab_pool.tile([128, NE], f32)
            nc.sync.dma_start(t_tab[:], tbl_d[:])
            t_ones = tab_pool.tile([128, 8], f32)
            nc.sync.dma_start(t_ones[:], ones_d[:])

            for g in range(NCHUNK):
                t_idx = idx_pool.tile([128, ICOL], i16, tag="idx")
                nc.sync.dma_start(t_idx[:], idx_d[:, g * ICOL:(g + 1) * ICOL])

                t_g = gath_pool.tile([128, NIDX], f32, tag="g")
                if "nogather" in ablate:
                    nc.gpsimd.memset(t_g[:, 0:4], 0.0)
                else:
                    nc.gpsimd.ap_gather(
                        t_g[:], t_tab[:], t_idx[:],
                        channels=128, num_elems=NE, d=1, num_idxs=NIDX,
                    )

                # centers, host-prereplicated: (128, HALF) per comp
                cc = []
                for c in range(3):
                    t_c = cen_pool.tile([128, HALF], f32, tag=f"cen{c}")
                    off = (c * NCHUNK + g) * HALF
                    nc.sync.dma_start(t_c[:], cen_d[:, off:off + HALF])
                    cc.append(t_c)

                # transpose gathered comps into slot layout (p=(i,s), f=atom)
                comps = []
                for c in range(3):
                    t_x = comp_pool.tile([128, HALF], f32, tag=f"comp{c}")
                    if "notrans" in ablate:
                        nc.gpsimd.memset(t_x[:, 0:4], 0.0)
                    else:
                        for k in range(NCORES):
                            row = 16 * k + 4 * c
                            src = t_g[row:row + 1, :].rearrange(
                                "one (p f) -> one p f", p=128)
                            nc.sync.dma_start(
                                t_x[:, 64 * k:64 * k + 64], src)
                    comps.append(t_x)

                # r2 = sum_c (xj - cx)^2, clamped to rc^2
                t_w = ew_pool.tile([128, HALF], f32, tag="w")
                t_d0 = ew_pool.tile([128, HALF], f32, tag="d0")
                nc.vector.tensor_tensor(t_d0[:], comps[0][:], cc[0][:],
                                        Alu.subtract)
                nc.vector.tensor_tensor(t_w[:], t_d0[:], t_d0[:], Alu.mult)
                for c in (1, 2):
                    t_dc = ew_pool.tile([128, HALF], f32, tag="d1")
                    nc.vector.tensor_tensor(t_dc[:], comps[c][:], cc[c][:],
                                            Alu.subtract)
                    t_sq = ew_pool.tile([128, HALF], f32, tag="sq")
                    nc.vector.tensor_tensor(t_sq[:], t_dc[:], t_dc[:],
                                            Alu.mult)
                    nc.vector.tensor_tensor(t_w[:], t_w[:], t_sq[:], Alu.add)
                nc.vector.tensor_scalar(t_w[:], t_w[:], rc2, None, Alu.min)

                # R = sqrt(w);  F = 0.5*sin(pi/rc * R + pi/2) + 0.5
                t_r = ew_pool.tile([128, HALF], f32, tag="r")
                nc.scalar.activation(t_r[:], t_w[:], AF.Sqrt)
                # F = 0.5(cos(pi R/rc)+1) = sin(pi/2 - pi R/(2rc))^2
                # keeps the Sin argument in [0, pi/2] and F == 0 at R == rc
                t_f = ew_pool.tile([128, HALF], f32, tag="f")
                nc.scalar.activation(t_f[:], t_r[:], AF.Sin,
                                     bias=float(np.pi / 2),
                                     scale=float(-np.pi / (2.0 * rc0)))
                # H = -e*w + 2*ln(max(s, tiny))
                nc.vector.tensor_scalar(t_f[:], t_f[:], 1.0e-30, None,
                                        Alu.max)
                t_lf = ew_pool.tile([128, HALF], f32, tag="lf")
                nc.scalar.activation(t_lf[:], t_f[:], AF.Ln)
                t_u = ew_pool.tile([128, HALF], f32, tag="u")
                nc.vector.tensor_scalar(t_u[:], t_w[:], -float(e0), None,
                                        Alu.mult)
                t_h = ew_pool.tile([128, HALF], f32, tag="h")
                nc.vector.scalar_tensor_tensor(
                    t_h[:], t_lf[:], 2.0, t_u[:],
                    op0=Alu.mult, op1=Alu.add)

                # psum columns: (qt, l, i, a); out row = atom-within-quarter
                t_psum = psum_pool.tile([128, HALF], f32, tag="ps")
                for l in range([0, L]["noll" not in ablate]):
                    rs_l = float(rs_vals[l])
                    t_q = k_pool.tile([128, HALF], f32, tag="q")
                    nc.vector.scalar_tensor_tensor(
                        t_q[:], t_r[:], 2.0 * float(e0) * rs_l, t_h[:],
                        op0=Alu.mult, op1=Alu.add)
                    t_k = k_pool.tile([128, HALF], f32, tag="k")
                    nc.scalar.activation(t_k[:], t_q[:], AF.Exp,
                                         bias=-float(e0) * rs_l * rs_l)
                    for qt in range(HALF // 128):
                        nc.tensor.matmul(
                            t_psum[:, qt * 128 + 8 * l:qt * 128 + 8 * l + 8],
                            t_k[:, qt * 128:(qt + 1) * 128],
                            t_ones[:, 0:8],
                            start=True, stop=True)

                t_o = out_pool.tile([128, HALF], f32, tag="o")
                if "noll" in ablate:
                    nc.gpsimd.memset(t_o[:], 0.0)
                else:
                    nc.vector.tensor_copy(t_o[:], t_psum[:])

                nc.sync.dma_start(
                    out_d[g * 128:(g + 1) * 128, :], t_o[:])

    nc.compile()
    return nc


def _host_prep(X, Nbrs, Nbrs_Z, atom_types):
    """Sort neighbors by type into padded slots; build per-core inputs."""
    tid_lut = np.full(256, 255, dtype=np.uint8)
    tid_lut[np.asarray(atom_types, dtype=np.int64)] = np.arange(A, dtype=np.uint8)
    tid = tid_lut[Nbrs_Z]                                   # (B,N,M)

    order = np.argsort(tid, axis=-1, kind="stable")
    tid_s = np.take_along_axis(tid, order, axis=-1)
    nbr_s = np.take_along_axis(Nbrs, order, axis=-1)

    counts = (tid[..., None, :] == np.arange(A, dtype=np.uint8)[:, None]
              ).sum(-1).astype(np.int32)                    # (B,N,A)
    starts = np.zeros((B, N, A), dtype=np.int32)
    starts[..., 1:] = np.cumsum(counts, axis=-1)[..., :-1]

    r = np.arange(PPT, dtype=np.int32)
    pos = starts[..., :, None] + r                          # (B,N,A,PPT)
    valid = r < counts[..., :, None]
    posc = np.minimum(pos, M - 1)
    slot_nbr = np.take_along_axis(
        nbr_s, posc.reshape(B, N, A * PPT), axis=-1).reshape(B, N, A, PPT)
    slot_idx = np.where(valid, slot_nbr, -1)                # (B,N,A,PPT)=-1 pad

    # dropped overflow elements (rank >= PPT within a type)
    drop_b, drop_n, drop_a = np.nonzero(counts > PPT)
    dropped = []
    for b, n, a in zip(drop_b, drop_n, drop_a):
        for rr in range(PPT, counts[b, n, a]):
            dropped.append((b, n, a, nbr_s[b, n, starts[b, n, a] + rr]))
    return slot_idx.reshape(B, N, S), dropped


def _host_correction(out, X, dropped, rc, rs, e):
    if not dropped:
        return
    d = np.asarray(dropped, dtype=np.int64)
    b, n, a, j = d[:, 0], d[:, 1], d[:, 2], d[:, 3]
    diff = X[b, j].astype(np.float64) - X[b, n].astype(np.float64)
    R = np.sqrt((diff * diff).sum(-1))
    rc64, rs64, e64 = (np.asarray(v, dtype=np.float64) for v in (rc, rs, e))
    K = np.exp(-e64[None, :] * (R[:, None] - rs64[None, :]) ** 2)
    FC = np.where(R[:, None] <= rc64[None, :],
                  0.5 * (np.cos(np.pi * R[:, None] / rc64[None, :]) + 1.0), 0.0)
    contrib = (K * FC).astype(np.float32)                   # (D, L)
    for i in range(len(b)):
        out[np.arange(L) * A + a[i], b[i], n[i]] += contrib[i]


def kernel(X, Nbrs, Nbrs_Z, rc, rs, e, atom_types):
    from concourse.bass_utils import run_bass_kernel_spmd

    X = np.asarray(X, dtype=np.float32)
    Nbrs = np.asarray(Nbrs, dtype=np.int32)
    Nbrs_Z = np.asarray(Nbrs_Z, dtype=np.int32)
    rc = np.asarray(rc, dtype=np.float32)
    rs = np.asarray(rs, dtype=np.float32)
    e = np.asarray(e, dtype=np.float32)
    atom_types = np.asarray(atom_types, dtype=np.int32)

    assert np.all(rc == rc[0]) and np.all(e == e[0]), \
        "fast path requires uniform rc and e"

    slot_idx, dropped = _host_prep(X, Nbrs, Nbrs_Z, atom_types)

    import os
    ablate = os.environ.get("KERNEL_ABLATE", "")
    key = (float(rc[0]), float(e[0]), tuple(np.round(rs.astype(float), 9)),
           ablate)
    if key not in _cache:
        _cache[key] = _build_program(float(rc[0]), float(e[0]),
                                     [float(v) for v in rs], ablate)
    nc = _cache[key]

    # block-ones lhsT: [p=(i,s), po=(i,a)] = (i==i') & (s//16==a)
    ones_np = np.zeros((128, 8), dtype=np.float32)
    for p in range(128):
        i, s = divmod(p, S)
        ones_np[p, i * 4 + s // PPT] = 1.0

    in_maps = []
    for core in range(NCORES):
        bs = core * BPC
        Xc = X[bs:bs + BPC]                                  # (BPC,N,3)
        sl = slot_idx[bs:bs + BPC].astype(np.int64)          # (BPC,N,S)
        flat = np.where(
            sl.reshape(BPC, N, S) >= 0,
            sl.reshape(BPC, N, S) +
            (np.arange(BPC, dtype=np.int64) * N)[:, None, None],
            SENT).reshape(AT, S)                             # atom-major

        tbl = np.full((128, NE), SENT_COORD, dtype=np.float32)
        coords = Xc.reshape(AT, 3)
        for k in range(NCORES):
            for c in range(3):
                tbl[16 * k + 4 * c, :AT] = coords[:, c]

        # index tiles: chunk g, core k covers f-block k; list order (p_dst, f)
        idx_np = np.zeros((128, NCHUNK * (NIDX // 16)), dtype=np.int16)
        ICOL = NIDX // 16
        for g in range(NCHUNK):
            # atoms of chunk: at = g*CH + i*HALF + f ; slots s
            at = (g * CH + np.arange(2)[:, None, None] * HALF
                  + np.arange(HALF)[None, None, :])          # (2,1,HALF)
            av = np.broadcast_to(at, (2, S, HALF))
            sv = np.broadcast_to(np.arange(S)[None, :, None], (2, S, HALF))
            vals = flat[av, sv].astype(np.int16)             # (2,S,HALF)=(p,f)
            vals = vals.reshape(128, NCORES, 64)             # f = 64k + fl
            for k in range(NCORES):
                lst = vals[:, k, :]                          # (p_dst=128, 64)
                # position j = p*64+fl -> partition 16k + j%16, col j//16
                wrapped = lst.reshape(128 * 64)
                part = 16 * k + (np.arange(128 * 64) % 16)
                col = np.arange(128 * 64) // 16
                tile = np.zeros((16, ICOL), dtype=np.int16)
                tile[part - 16 * k, col] = wrapped
                idx_np[16 * k:16 * k + 16, g * ICOL:(g + 1) * ICOL] = tile

        # centers replicated into slot layout: (128,HALF) per comp per chunk
        cen_np = np.zeros((128, 3 * NCHUNK * HALF), dtype=np.float32)
        for c in range(3):
            for g in range(NCHUNK):
                at0 = g * CH
                row = coords[at0:at0 + CH, c].reshape(2, HALF)  # (i, f)
                blk = np.repeat(row[:, None, :], S, axis=1).reshape(128, HALF)
                off = (c * NCHUNK + g) * HALF
                cen_np[:, off:off + HALF] = blk

        in_maps.append({"tbl": tbl, "idx": idx_np, "cen": cen_np,
                        "ones": ones_np})

    res = run_bass_kernel_spmd(nc, in_maps, core_ids=list(range(NCORES)))
    global _last_nc, _last_in_maps
    _last_nc, _last_in_maps = nc, in_maps

    out = np.empty((L * A, B, N), dtype=np.float32)
    for core in range(NCORES):
        # raw[g, f, qt, l, i, a] = sym[l*4+a, b=g//2, (g%2)*1024+i*512+qt*128+f]
        raw = res.results[core]["out"].reshape(BPC, 2, 128, 4, L, 2, A)
        perm = raw.transpose(4, 6, 0, 1, 5, 3, 2)   # (l,a,b,h2,i,qt,f)
        out[:, core * BPC:(core + 1) * BPC, :] = perm.reshape(L * A, BPC, N)

    _host_correction(out, X, dropped, rc, rs, e)
    return out


def benchmark(n_iters=20):
    """Steady-state wall time per execution of the compiled program."""
    import time
    import jax
    import numpy as np
    from jax.sharding import Mesh, PartitionSpec
    from jax.experimental.shard_map import shard_map
    from concourse import mybir
    from concourse.bass2jax import (_bass_exec_p, install_neuronx_cc_hook,
                                    partition_id_tensor)

    nc, in_maps = _last_nc, _last_in_maps
    install_neuronx_cc_hook()
    partition_name = (nc.partition_id_tensor.name
                      if nc.partition_id_tensor else None)
    in_names, out_names, out_avals, zero_outs = [], [], [], []
    for alloc in nc.m.functions[0].allocations:
        if not isinstance(alloc, mybir.MemoryLocationSet):
            continue
        name = alloc.memorylocations[0].name
        if alloc.kind == "ExternalInput":
            if name != partition_name:
                in_names.append(name)
        elif alloc.kind == "ExternalOutput":
            shape = tuple(alloc.tensor_shape)
            dtype = mybir.dt.np(alloc.dtype)
            out_names.append(name)
            out_avals.append(jax.core.ShapedArray(shape, dtype))
            zero_outs.append(np.zeros(shape, dtype))
    n_params = len(in_names)
    all_in_names = in_names + out_names + (
        [partition_name] if partition_name else [])

    n_out = len(out_names)

    def _mk_body(reps):
        def _body(*args):
            ins = list(args[:n_params])
            outs = list(args[n_params:])
            for _ in range(reps):
                operands = ins + outs
                if partition_name is not None:
                    operands.append(partition_id_tensor())
                outs = list(_bass_exec_p.bind(
                    *operands, out_avals=tuple(out_avals),
                    in_names=tuple(all_in_names), out_names=tuple(out_names),
                    lowering_input_output_aliases=(),
                    sim_require_finite=True, sim_require_nnan=True, nc=nc))
            return tuple(outs)
        return _body
    _body = _mk_body(1)

    devices = jax.devices()[:NCORES]
    mesh = Mesh(np.asarray(devices), ("core",))
    nin = n_params + len(out_names)

    def _jit(reps):
        return jax.jit(shard_map(
            _mk_body(reps), mesh=mesh,
            in_specs=(PartitionSpec("core"),) * nin,
            out_specs=(PartitionSpec("core"),) * len(out_names),
            check_rep=False), keep_unused=True)

    concat_in = [np.concatenate([np.asarray(m[name]) for m in in_maps], axis=0)
                 for name in in_names]
    concat_zeros = [np.zeros((NCORES * z.shape[0], *z.shape[1:]), z.dtype)
                    for z in zero_outs]
    from jax.sharding import NamedSharding
    sh = NamedSharding(mesh, PartitionSpec("core"))
    args = [jax.device_put(a, sh) for a in concat_in + concat_zeros]

    def _time(fn):
        out = fn(*args)
        jax.block_until_ready(out)
        ts = []
        for _ in range(n_iters):
            t0 = time.perf_counter()
            out = fn(*args)
            jax.block_until_ready(out)
            ts.append(time.perf_counter() - t0)
        return min(ts)

    REPS = 5
    t1 = _time(_jit(1))
    tR = _time(_jit(REPS))
    per_exec = (tR - t1) / (REPS - 1)
    return per_exec, t1



# revision 2
# speedup vs baseline: 38.7679x; 38.7679x over previous
"""AtomicConv radial symmetry function kernel for Trainium2 (8 NeuronCores).

Strategy (v2):
  - Data-parallel over batch: 4 examples per core (8192 atoms per core).
  - Host sorts each atom's 64 neighbors by atom-type and keeps the typed
    ones, padded at the ATOM level to SA=36 slots (excess beyond 36 is
    host-corrected; P ~ 0.04% of atoms). Pad slots point at table index 0
    and are killed by host-shipped per-(slot,atom) type-mask planes, so no
    sentinel coordinates are needed and the per-type padding waste of the
    old 4x16 layout is gone (64 -> 36 gather indices per atom).
  - Device per chunk of 1024 atoms (2 halves x 512):
      gpsimd.ap_gather fetches neighbor x/y/z (per-Q7-core index lists,
      components as table rows 16k+4c) -> SBUF->SBUF DMAs transpose into
      the slot grid (p = h*36+s, f = atom) -> DVE computes clamped R^2,
      ACT computes R, the cosine window F and ln(F); per radial filter l
      the Gaussian is exp(a_l*R + h + b_l) with h = -e*R^2 + 2 ln F ->
      DVE multiplies by the 4 type-mask planes -> PE contracts the 36
      slot partitions per (l, a, half) into PSUM (atoms on the psum
      partition dim) -> one drain + one store per chunk.
  - The gather is the bottleneck (~34 ns per Q7-core index, latency-bound
    in the Q7 SBUF read commands); all other engines hide under it.
"""

import numpy as np

B, N, M = 32, 2048, 64
L, A = 16, 4
NCORES = 8
BPC = B // NCORES          # examples per core
AT = BPC * N               # atoms per core (8192)
SA = 36                    # slots per atom (atom-level padding)
CH = 1024                  # atoms per chunk
HALF = CH // 2             # 512 atoms per half -> free dim
NCHUNK = AT // CH          # 8
PUSE = 2 * SA              # used partitions in the slot grid (72)
NIDX = SA * CH // NCORES   # gathered idxs per Q7 core per chunk (4608)
ICOL = NIDX // 16          # idx cols per chunk (288)
FPC = HALF // NCORES       # atoms (free positions) per Q7 core (64)
NE = AT                    # gather table entries
NQT = HALF // 128          # psum partition blocks per half (4)

_cache = {}


def _build_program(rc0, e0, rs_vals, ablate=""):
    import concourse.bacc as bacc
    import concourse.mybir as mybir
    from concourse.tile import TileContext

    f32 = mybir.dt.float32
    i16 = mybir.dt.int16
    AF = mybir.ActivationFunctionType
    Alu = mybir.AluOpType

    nc = bacc.Bacc(None, target_bir_lowering=False)

    # register const APs for every activation bias value we use
    bias_vals = {float(np.pi / 2), 1.0e-38}
    for l in range(L):
        bias_vals.add(-float(e0) * float(rs_vals[l]) * float(rs_vals[l]))
    for v in sorted(bias_vals):
        if (f32, v) not in nc.const_aps.aps:
            t = nc.alloc_sbuf_tensor(f"cst-{v!r}", [128, 1], f32)
            nc.gpsimd.memset(t.ap(), v)
            nc.const_aps.aps[(f32, v)] = t.ap()
    nc.all_engine_barrier()

    tbl_d = nc.dram_tensor("tbl", (128, NE), f32, kind="ExternalInput")
    idx_d = nc.dram_tensor("idx", (128, NCHUNK * ICOL), i16,
                           kind="ExternalInput")
    cen_d = nc.dram_tensor("cen", (128, 3 * NCHUNK * HALF), f32,
                           kind="ExternalInput")
    msk_d = nc.dram_tensor("msk", (128, NCHUNK * A * HALF), f32,
                           kind="ExternalInput")
    hsel_d = nc.dram_tensor("hsel", (128, 2), f32, kind="ExternalInput")
    out_d = nc.dram_tensor("out", (NCHUNK * 128, HALF), f32,
                           kind="ExternalOutput")

    rc2 = float(rc0) * float(rc0)

    with TileContext(nc) as tc:
        with (
            tc.tile_pool(name="tab", bufs=1) as tab_pool,
            tc.tile_pool(name="gath", bufs=2) as gath_pool,
            tc.tile_pool(name="idxp", bufs=2) as idx_pool,
            tc.tile_pool(name="cenp", bufs=2) as cen_pool,
            tc.tile_pool(name="mskp", bufs=2) as msk_pool,
            tc.tile_pool(name="comp", bufs=2) as comp_pool,
            tc.tile_pool(name="ew", bufs=2) as ew_pool,
            tc.tile_pool(name="kt", bufs=3) as k_pool,
            tc.tile_pool(name="pt", bufs=3) as p_pool,
            tc.tile_pool(name="ot", bufs=2) as out_pool,
            tc.tile_pool(name="ps", bufs=2, space="PSUM") as psum_pool,
        ):
            t_tab = tab_pool.tile([128, NE], f32)
            nc.sync.dma_start(t_tab[:], tbl_d[:])
            t_h = tab_pool.tile([128, 2], f32)
            nc.sync.dma_start(t_h[:], hsel_d[:])

            for g in range(NCHUNK):
                t_idx = idx_pool.tile([128, ICOL], i16, tag="idx")
                nc.sync.dma_start(t_idx[:], idx_d[:, g * ICOL:(g + 1) * ICOL])

                t_g = gath_pool.tile([128, NIDX], f32, tag="g")
                if "nogather" in ablate:
                    nc.gpsimd.memset(t_g[:, 0:4], 0.0)
                else:
                    nc.gpsimd.ap_gather(
                        t_g[:], t_tab[:], t_idx[:],
                        channels=128, num_elems=NE, d=1, num_idxs=NIDX,
                    )

                # centers + masks (host-prereplicated slot-grid layouts)
                cc = []
                for c in range(3):
                    t_c = cen_pool.tile([128, HALF], f32, tag=f"cen{c}")
                    off = (c * NCHUNK + g) * HALF
                    nc.sync.dma_start(t_c[:PUSE], cen_d[:PUSE, off:off + HALF])
                    cc.append(t_c)
                t_m = msk_pool.tile([128, A * HALF], f32, tag="msk")
                nc.sync.dma_start(
                    t_m[:PUSE], msk_d[:PUSE, g * A * HALF:(g + 1) * A * HALF])

                # transpose gathered comps into slot grid (p=(h,s), f=atom)
                comps = []
                for c in range(3):
                    t_x = comp_pool.tile([128, HALF], f32, tag=f"comp{c}")
                    if "notrans" in ablate:
                        nc.gpsimd.memset(t_x[:, 0:4], 0.0)
                    else:
                        for k in range(NCORES):
                            row = 16 * k + 4 * c
                            src = t_g[row:row + 1, :].rearrange(
                                "one (p f) -> one p f", p=PUSE)
                            nc.sync.dma_start(
                                t_x[:PUSE, FPC * k:FPC * k + FPC], src)
                    comps.append(t_x)

                # r2 = sum_c (xj - cx)^2, clamped to rc^2
                t_w = ew_pool.tile([128, HALF], f32, tag="w")
                t_d0 = ew_pool.tile([128, HALF], f32, tag="d0")
                nc.vector.tensor_tensor(t_d0[:PUSE], comps[0][:PUSE],
                                        cc[0][:PUSE], Alu.subtract)
                nc.vector.tensor_tensor(t_w[:PUSE], t_d0[:PUSE], t_d0[:PUSE],
                                        Alu.mult)
                for c in (1, 2):
                    t_dc = ew_pool.tile([128, HALF], f32, tag="d1")
                    nc.vector.tensor_tensor(t_dc[:PUSE], comps[c][:PUSE],
                                            cc[c][:PUSE], Alu.subtract)
                    t_sq = ew_pool.tile([128, HALF], f32, tag="sq")
                    nc.vector.tensor_tensor(t_sq[:PUSE], t_dc[:PUSE],
                                            t_dc[:PUSE], Alu.mult)
                    nc.vector.tensor_tensor(t_w[:PUSE], t_w[:PUSE],
                                            t_sq[:PUSE], Alu.add)
                nc.vector.tensor_scalar(t_w[:PUSE], t_w[:PUSE], rc2, None,
                                        Alu.min)

                # R = sqrt(w);  F = sin(pi/2 - pi R/(2rc)) so F^2 is the
                # cosine window; h = -e*w + 2*ln(max(F, tiny))
                t_r = ew_pool.tile([128, HALF], f32, tag="r")
                nc.scalar.activation(t_r[:PUSE], t_w[:PUSE], AF.Sqrt)
                t_f = ew_pool.tile([128, HALF], f32, tag="f")
                nc.scalar.activation(t_f[:PUSE], t_r[:PUSE], AF.Sin,
                                     bias=float(np.pi / 2),
                                     scale=float(-np.pi / (2.0 * rc0)))
                nc.vector.tensor_scalar(t_f[:PUSE], t_f[:PUSE], 1.0e-30,
                                        None, Alu.max)
                t_lf = ew_pool.tile([128, HALF], f32, tag="lf")
                nc.scalar.activation(t_lf[:PUSE], t_f[:PUSE], AF.Ln)
                t_u = ew_pool.tile([128, HALF], f32, tag="u")
                nc.vector.tensor_scalar(t_u[:PUSE], t_w[:PUSE], -float(e0),
                                        None, Alu.mult)
                t_hh = ew_pool.tile([128, HALF], f32, tag="h")
                nc.vector.scalar_tensor_tensor(
                    t_hh[:PUSE], t_lf[:PUSE], 2.0, t_u[:PUSE],
                    op0=Alu.mult, op1=Alu.add)

                # psum: po = atom-in-qt-block, col = qt*128 + (l*4+a)*2 + h
                t_psum = psum_pool.tile([128, HALF], f32, tag="ps")
                for l in range([0, L]["noll" not in ablate]):
                    rs_l = float(rs_vals[l])
                    t_q = k_pool.tile([128, HALF], f32, tag="q")
                    nc.vector.scalar_tensor_tensor(
                        t_q[:PUSE], t_r[:PUSE], 2.0 * float(e0) * rs_l,
                        t_hh[:PUSE], op0=Alu.mult, op1=Alu.add)
                    t_k = k_pool.tile([128, HALF], f32, tag="k")
                    nc.scalar.activation(t_k[:PUSE], t_q[:PUSE], AF.Exp,
                                         bias=-float(e0) * rs_l * rs_l)
                    for a in range(A):
                        t_p = p_pool.tile([128, HALF], f32, tag="p")
                        nc.vector.tensor_tensor(
                            t_p[:PUSE], t_k[:PUSE],
                            t_m[:PUSE, a * HALF:(a + 1) * HALF], Alu.mult)
                        col = (l * A + a) * 2
                        for qt in range(NQT):
                            nc.tensor.matmul(
                                t_psum[:, qt * 128 + col:qt * 128 + col + 2],
                                t_p[:PUSE, qt * 128:(qt + 1) * 128],
                                t_h[:PUSE, 0:2],
                                start=True, stop=True)

                t_o = out_pool.tile([128, HALF], f32, tag="o")
                if "noll" in ablate:
                    nc.gpsimd.memset(t_o[:], 0.0)
                else:
                    nc.vector.tensor_copy(t_o[:], t_psum[:])

                nc.sync.dma_start(
                    out_d[g * 128:(g + 1) * 128, :], t_o[:])

    nc.compile()
    return nc


def _host_prep(X, Nbrs, Nbrs_Z, atom_types):
    """Type-sort neighbors, pad per atom to SA slots; build per-core inputs.

    Returns slot_idx (B,N,SA) local-to-example neighbor ids (0 for pads),
    slot_type (B,N,SA) type id (255 for pads), and the dropped pairs
    (atoms with more than SA typed neighbors)."""
    tid_lut = np.full(256, 255, dtype=np.uint8)
    tid_lut[np.asarray(atom_types, dtype=np.int64)] = np.arange(
        A, dtype=np.uint8)
    tid = tid_lut[Nbrs_Z]                                   # (B,N,M)

    order = np.argsort(tid, axis=-1, kind="stable")
    tid_s = np.take_along_axis(tid, order, axis=-1)         # (B,N,M)
    nbr_s = np.take_along_axis(Nbrs, order, axis=-1)

    typed = tid_s != 255
    slot_idx = np.where(typed[..., :SA], nbr_s[..., :SA], 0).astype(np.int32)
    slot_type = np.where(typed[..., :SA], tid_s[..., :SA], 255).astype(
        np.uint8)

    over = typed[..., SA:]
    drop_b, drop_n, drop_m = np.nonzero(over)
    drop_a = tid_s[drop_b, drop_n, drop_m + SA].astype(np.int64)
    drop_j = nbr_s[drop_b, drop_n, drop_m + SA].astype(np.int64)
    return slot_idx, slot_type, (drop_b, drop_n, drop_a, drop_j)


def _host_correction(out, X, drops, rc, rs, e):
    b, n, a, j = drops
    if len(b) == 0:
        return
    diff = X[b, j].astype(np.float64) - X[b, n].astype(np.float64)
    R = np.sqrt((diff * diff).sum(-1))                      # (D,)
    rc64, rs64, e64 = (np.asarray(v, dtype=np.float64) for v in (rc, rs, e))
    K = np.exp(-e64[None, :] * (R[:, None] - rs64[None, :]) ** 2)
    FC = np.where(R[:, None] <= rc64[None, :],
                  0.5 * (np.cos(np.pi * R[:, None] / rc64[None, :]) + 1.0),
                  0.0)
    contrib = (K * FC)                                      # (D, L)
    la = (np.arange(L)[None, :] * A + a[:, None])           # (D, L)
    flat = out.reshape(L * A, B * N)
    np.add.at(flat, (la.ravel(),
                     np.repeat(b * N + n, L)),
              contrib.astype(np.float32).ravel())


def kernel(X, Nbrs, Nbrs_Z, rc, rs, e, atom_types):
    from concourse.bass_utils import run_bass_kernel_spmd

    X = np.asarray(X, dtype=np.float32)
    Nbrs = np.asarray(Nbrs, dtype=np.int32)
    Nbrs_Z = np.asarray(Nbrs_Z, dtype=np.int32)
    rc = np.asarray(rc, dtype=np.float32)
    rs = np.asarray(rs, dtype=np.float32)
    e = np.asarray(e, dtype=np.float32)
    atom_types = np.asarray(atom_types, dtype=np.int32)

    assert np.all(rc == rc[0]) and np.all(e == e[0]), \
        "fast path requires uniform rc and e"

    slot_idx, slot_type, drops = _host_prep(X, Nbrs, Nbrs_Z, atom_types)

    import os
    ablate = os.environ.get("KERNEL_ABLATE", "")
    key = (float(rc[0]), float(e[0]), tuple(np.round(rs.astype(float), 9)),
           ablate)
    if key not in _cache:
        _cache[key] = _build_program(float(rc[0]), float(e[0]),
                                     [float(v) for v in rs], ablate)
    nc = _cache[key]

    # h-selector for the PE contraction: rows (h*SA+s) -> column h
    hsel_np = np.zeros((128, 2), dtype=np.float32)
    for h in range(2):
        hsel_np[h * SA:(h + 1) * SA, h] = 1.0

    in_maps = []
    for core in range(NCORES):
        bs = core * BPC
        coords = X[bs:bs + BPC].reshape(AT, 3)               # local atoms
        # local (per-core) neighbor ids: example offset folded in
        sl = (slot_idx[bs:bs + BPC].astype(np.int64)
              + (np.arange(BPC, dtype=np.int64) * N)[:, None, None]
              ).reshape(AT, SA)
        st = slot_type[bs:bs + BPC].reshape(AT, SA)

        tbl = np.zeros((128, NE), dtype=np.float32)
        for k in range(NCORES):
            for c in range(3):
                tbl[16 * k + 4 * c, :] = coords[:, c]

        # index tiles: chunk g, Q7 core k covers f-block k.
        # list position j = (h*SA+s)*FPC + fl ; atom = g*CH + h*HALF + k*FPC + fl
        idx_np = np.zeros((128, NCHUNK * ICOL), dtype=np.int16)
        hh = np.arange(2)[:, None, None]
        ss = np.arange(SA)[None, :, None]
        fl = np.arange(FPC)[None, None, :]
        for g in range(NCHUNK):
            for k in range(NCORES):
                at = g * CH + hh * HALF + k * FPC + fl       # (2,SA,FPC)
                av = np.broadcast_to(at, (2, SA, FPC))
                sv = np.broadcast_to(ss, (2, SA, FPC))
                vals = sl[av, sv].astype(np.int16).reshape(2 * SA * FPC)
                part = 16 * k + (np.arange(2 * SA * FPC) % 16)
                col = np.arange(2 * SA * FPC) // 16
                tile = np.zeros((16, ICOL), dtype=np.int16)
                tile[part - 16 * k, col] = vals
                idx_np[16 * k:16 * k + 16, g * ICOL:(g + 1) * ICOL] = tile

        # centers replicated into the slot grid: (PUSE, HALF) per comp/chunk
        cen_np = np.zeros((128, 3 * NCHUNK * HALF), dtype=np.float32)
        for c in range(3):
            for g in range(NCHUNK):
                row = coords[g * CH:(g + 1) * CH, c].reshape(2, HALF)
                blk = np.repeat(row[:, None, :], SA, axis=1).reshape(
                    PUSE, HALF)
                off = (c * NCHUNK + g) * HALF
                cen_np[:PUSE, off:off + HALF] = blk

        # type-mask planes: (PUSE, A*HALF) per chunk
        msk_np = np.zeros((128, NCHUNK * A * HALF), dtype=np.float32)
        for g in range(NCHUNK):
            stc = st[g * CH:(g + 1) * CH].reshape(2, HALF, SA)
            grid = stc.transpose(0, 2, 1).reshape(PUSE, HALF)  # (h*SA+s, f)
            for a in range(A):
                off = (g * A + a) * HALF
                msk_np[:PUSE, off:off + HALF] = (grid == a)

        in_maps.append({"tbl": tbl, "idx": idx_np, "cen": cen_np,
                        "msk": msk_np, "hsel": hsel_np})

    res = run_bass_kernel_spmd(nc, in_maps, core_ids=list(range(NCORES)))
    global _last_nc, _last_in_maps
    _last_nc, _last_in_maps = nc, in_maps

    out = np.empty((L * A, B, N), dtype=np.float32)
    for core in range(NCORES):
        # raw[g, po, qt, la, h] -> atom = g*CH + h*HALF + qt*128 + po
        raw = res.results[core]["out"].reshape(NCHUNK, 128, NQT, L * A, 2)
        perm = raw.transpose(3, 0, 4, 2, 1)     # (la, g, h, qt, po)
        out[:, core * BPC:(core + 1) * BPC, :] = perm.reshape(
            L * A, BPC, N)

    _host_correction(out, X, drops, rc, rs, e)
    return out


def benchmark(n_pairs=14, klo=2, khi=22):
    """Steady-state per-execution device time of the compiled program.

    Chains k executions of the single jitted bass program (async dispatch
    pipelines them on the device) and takes the median of paired slopes
    (t_khi - t_klo) / (khi - klo), which cancels the per-call dispatch
    overhead of the remote tunnel."""
    import time
    import jax
    import numpy as np
    from jax.sharding import Mesh, PartitionSpec, NamedSharding
    from jax.experimental.shard_map import shard_map
    from concourse import mybir
    from concourse.bass2jax import (_bass_exec_p, install_neuronx_cc_hook,
                                    partition_id_tensor)

    nc, in_maps = _last_nc, _last_in_maps
    install_neuronx_cc_hook()
    partition_name = (nc.partition_id_tensor.name
                      if nc.partition_id_tensor else None)
    in_names, out_names, out_avals, zero_outs = [], [], [], []
    for alloc in nc.m.functions[0].allocations:
        if not isinstance(alloc, mybir.MemoryLocationSet):
            continue
        name = alloc.memorylocations[0].name
        if alloc.kind == "ExternalInput":
            if name != partition_name:
                in_names.append(name)
        elif alloc.kind == "ExternalOutput":
            shape = tuple(alloc.tensor_shape)
            dtype = mybir.dt.np(alloc.dtype)
            out_names.append(name)
            out_avals.append(jax.core.ShapedArray(shape, dtype))
            zero_outs.append(np.zeros(shape, dtype))
    n_params = len(in_names)
    all_in_names = in_names + out_names + (
        [partition_name] if partition_name else [])

    def _body(*args):
        ins = list(args[:n_params])
        outs = list(args[n_params:])
        operands = ins + outs
        if partition_name is not None:
            operands.append(partition_id_tensor())
        outs = list(_bass_exec_p.bind(
            *operands, out_avals=tuple(out_avals),
            in_names=tuple(all_in_names), out_names=tuple(out_names),
            lowering_input_output_aliases=(),
            sim_require_finite=True, sim_require_nnan=True, nc=nc))
        return tuple(outs)

    devices = jax.devices()[:NCORES]
    mesh = Mesh(np.asarray(devices), ("core",))
    fn = jax.jit(shard_map(
        _body, mesh=mesh,
        in_specs=(PartitionSpec("core"),) * (n_params + len(out_names)),
        out_specs=(PartitionSpec("core"),) * len(out_names),
        check_rep=False), keep_unused=True)
    concat_in = [np.concatenate([np.asarray(m[nm]) for m in in_maps], axis=0)
                 for nm in in_names]
    concat_zeros = [np.zeros((NCORES * z.shape[0], *z.shape[1:]), z.dtype)
                    for z in zero_outs]
    sh = NamedSharding(mesh, PartitionSpec("core"))
    ins_dev = [jax.device_put(a, sh) for a in concat_in]
    outs_dev = tuple(jax.device_put(a, sh) for a in concat_zeros)

    def chain(k):
        outs = outs_dev
        for _ in range(k):
            outs = fn(*ins_dev, *outs)
        jax.block_until_ready(outs)

    chain(1)
    slopes = []
    for _ in range(n_pairs):
        t0 = time.perf_counter(); chain(klo); a = time.perf_counter() - t0
        t0 = time.perf_counter(); chain(khi); b = time.perf_counter() - t0
        slopes.append((b - a) / (khi - klo))
    slopes = np.array(slopes)
    return float(np.median(slopes)), float(np.percentile(slopes, 75))


# revision 6
# speedup vs baseline: 47.1328x; 1.2158x over previous
"""AtomicConv radial symmetry function kernel for Trainium2 (8 NeuronCores).

Strategy (v3):
  - Data-parallel over batch: 4 examples per core (8192 atoms per core).
  - Host sorts each atom's 64 neighbors by atom-type and keeps the typed
    ones, padded at the ATOM level to SA=36 slots (excess beyond 36 is
    host-corrected; ~0.04% of atoms). Pad slots point at table index 0 and
    are killed by type masks computed on device from a compact bf16
    type-grid, so the old per-type 4x16 slot padding (64 gather indices
    per atom) shrinks to 36.
  - Device per chunk of 1024 atoms (2 halves x 512 on the free dim, slot
    grid p = h*36+s on partitions):
      gpsimd.ap_gather fetches neighbor x/y/z (per-Q7-core index lists,
      components as table rows 16k+4c) -> one strided SBUF->SBUF DMA per
      component transposes into the slot grid -> chunk centers are
      broadcast across the 36 slot rows with a tiny table-slice DMA + PE
      selector matmul (no replicated center upload) -> DVE computes
      clamped R^2 (subtracting centers straight out of PSUM), ACT computes
      R, the cosine window F and ln F; per radial filter l the Gaussian is
      exp(a_l*R + h + b_l) with h = -e*R^2 + 2 ln F, written in bf16 ->
      DVE multiplies by the 4 bf16 type-mask planes (2x mode) -> PE
      contracts the 72 slot-grid partitions per (l, a) with an h-selector
      into PSUM (atoms on the psum partition dim) -> one drain + one
      store per chunk.
  - DMAs are split between the two HWDGE queues (SP + Activation issue
    engines); the gather (~34 ns per Q7-core index, SBUF read-command
    latency bound) is the bottleneck and everything else hides under it.
"""

import numpy as np

B, N, M = 32, 2048, 64
L, A = 16, 4
NCORES = 8
BPC = B // NCORES          # examples per core
AT = BPC * N               # atoms per core (8192)
SA = 36                    # slots per atom (atom-level padding)
CH = 1024                  # atoms per chunk
HALF = CH // 2             # 512 atoms per half -> free dim
NCHUNK = AT // CH          # 8
PUSE = 2 * SA              # used partitions in the slot grid (72)
NIDX = SA * CH // NCORES   # gathered idxs per Q7 core per chunk (4608)
ICOL = NIDX // 16          # idx cols per chunk (288)
FPC = HALF // NCORES       # atoms (free positions) per Q7 core (64)
NE = AT                    # gather table entries
NQT = HALF // 128          # psum partition blocks per half (4)

_cache = {}


def _build_program(rc0, e0, rs_vals, ablate=""):
    import concourse.bacc as bacc
    import concourse.mybir as mybir
    from concourse.tile import TileContext

    f32 = mybir.dt.float32
    bf16 = mybir.dt.bfloat16
    i16 = mybir.dt.int16
    AF = mybir.ActivationFunctionType
    Alu = mybir.AluOpType

    nc = bacc.Bacc(None, target_bir_lowering=False)

    # register const APs for every activation bias value we use
    bias_vals = {float(np.pi / 2), 1.0e-38}
    for l in range(L):
        bias_vals.add(-float(e0) * float(rs_vals[l]) * float(rs_vals[l]))
    for v in sorted(bias_vals):
        if (f32, v) not in nc.const_aps.aps:
            t = nc.alloc_sbuf_tensor(f"cst-{v!r}", [128, 1], f32)
            nc.gpsimd.memset(t.ap(), v)
            nc.const_aps.aps[(f32, v)] = t.ap()
    nc.all_engine_barrier()

    tbl_d = nc.dram_tensor("tbl", (32, NE), f32, kind="ExternalInput")
    idx_d = nc.dram_tensor("idx", (128, NCHUNK * ICOL), i16,
                           kind="ExternalInput")
    grid_d = nc.dram_tensor("grid", (128, NCHUNK * HALF), bf16,
                            kind="ExternalInput")
    hsel_d = nc.dram_tensor("hsel", (128, 2), bf16, kind="ExternalInput")
    hsel2_d = nc.dram_tensor("hsel2", (128, PUSE), f32, kind="ExternalInput")
    out_d = nc.dram_tensor("out", (NCHUNK * 128, HALF), f32,
                           kind="ExternalOutput")

    rc2 = float(rc0) * float(rc0)

    with TileContext(nc) as tc:
        with (
            tc.tile_pool(name="tab", bufs=1) as tab_pool,
            tc.tile_pool(name="gath", bufs=3) as gath_pool,
            tc.tile_pool(name="idxp", bufs=2) as idx_pool,
            tc.tile_pool(name="grdp", bufs=2) as grd_pool,
            tc.tile_pool(name="ccp", bufs=2) as cc_pool,
            tc.tile_pool(name="mskp", bufs=2) as msk_pool,
            tc.tile_pool(name="comp", bufs=2) as comp_pool,
            tc.tile_pool(name="ew", bufs=2) as ew_pool,
            tc.tile_pool(name="kt", bufs=3) as k_pool,
            tc.tile_pool(name="pt", bufs=3) as p_pool,
            tc.tile_pool(name="ot", bufs=2) as out_pool,
            tc.tile_pool(name="ps", bufs=2, space="PSUM") as psum_pool,
            tc.tile_pool(name="psc", bufs=2, space="PSUM") as psc_pool,
        ):
            # table rows: partition 4*(4k+c) holds component c for Q7 core k
            t_tab = tab_pool.tile([128, NE], f32)
            nc.sync.dma_start(t_tab[0::4, :], tbl_d[:])
            t_h = tab_pool.tile([128, 2], bf16)
            nc.sync.dma_start(t_h[:], hsel_d[:])
            t_h2 = tab_pool.tile([128, PUSE], f32)
            nc.sync.dma_start(t_h2[:], hsel2_d[:])

            for g in range(NCHUNK):
                t_idx = idx_pool.tile([128, ICOL], i16, tag="idx")
                nc.sync.dma_start(t_idx[:], idx_d[:, g * ICOL:(g + 1) * ICOL])
                t_grid = grd_pool.tile([128, HALF], bf16, tag="grid")
                nc.scalar.dma_start(
                    t_grid[:PUSE], grid_d[:PUSE, g * HALF:(g + 1) * HALF])

                t_g = gath_pool.tile([128, NIDX], f32, tag="g")
                if "nogather" in ablate:
                    nc.gpsimd.memset(t_g[:, 0:4], 0.0)
                else:
                    nc.gpsimd.ap_gather(
                        t_g[:], t_tab[:], t_idx[:],
                        channels=128, num_elems=NE, d=1, num_idxs=NIDX,
                    )

                # chunk centers: tiny table-slice DMA to [2, HALF], then
                # PE broadcast across the 36 slot rows -> psum [PUSE, HALF]
                cenps = []
                for c in range(3):
                    t_cc = cc_pool.tile([128, HALF], f32, tag=f"cc{c}")
                    src = t_tab[4 * c:4 * c + 1,
                                g * CH:(g + 1) * CH].rearrange(
                        "one (h f) -> one h f", h=2)
                    eng = nc.scalar if c != 1 else nc.sync
                    eng.dma_start(t_cc[0:2, :], src)
                    t_pc = psc_pool.tile([128, HALF], f32, tag=f"cps{c}")
                    nc.tensor.matmul(t_pc[0:PUSE, :], t_h2[0:2, 0:PUSE],
                                     t_cc[0:2, :], start=True, stop=True)
                    cenps.append(t_pc)

                # type-mask planes from the compact grid
                masks = []
                for a in range(A):
                    t_m = msk_pool.tile([128, HALF], bf16, tag=f"m{a}")
                    nc.vector.tensor_scalar(t_m[:PUSE], t_grid[:PUSE],
                                            float(a), None, Alu.is_equal)
                    masks.append(t_m)

                # transpose gathered comps into slot grid (p=(h,s), f=atom)
                comps = []
                for c in range(3):
                    t_x = comp_pool.tile([128, HALF], f32, tag=f"comp{c}")
                    if "notrans" in ablate:
                        nc.gpsimd.memset(t_x[:, 0:4], 0.0)
                    else:
                        for k in range(NCORES):
                            row = 16 * k + 4 * c
                            src = t_g[row:row + 1, :].rearrange(
                                "one (p f) -> one p f", p=PUSE)
                            eng = nc.sync if (c + k) % 2 else nc.scalar
                            eng.dma_start(
                                t_x[:PUSE, FPC * k:FPC * k + FPC], src)
                    comps.append(t_x)

                # r2 = sum_c (xj - cx)^2 (centers read from psum), clamped
                t_w = ew_pool.tile([128, HALF], f32, tag="w")
                t_d0 = ew_pool.tile([128, HALF], f32, tag="d0")
                nc.vector.tensor_tensor(t_d0[:PUSE], comps[0][:PUSE],
                                        cenps[0][:PUSE], Alu.subtract)
                nc.vector.tensor_tensor(t_w[:PUSE], t_d0[:PUSE], t_d0[:PUSE],
                                        Alu.mult)
                for c in (1, 2):
                    t_dc = ew_pool.tile([128, HALF], f32, tag="d1")
                    nc.vector.tensor_tensor(t_dc[:PUSE], comps[c][:PUSE],
                                            cenps[c][:PUSE], Alu.subtract)
                    t_sq = ew_pool.tile([128, HALF], f32, tag="sq")
                    nc.vector.tensor_tensor(t_sq[:PUSE], t_dc[:PUSE],
                                            t_dc[:PUSE], Alu.mult)
                    nc.vector.tensor_tensor(t_w[:PUSE], t_w[:PUSE],
                                            t_sq[:PUSE], Alu.add)
                nc.vector.tensor_scalar(t_w[:PUSE], t_w[:PUSE], rc2, None,
                                        Alu.min)

                # R = sqrt(w);  F = sin(pi/2 - pi R/(2rc)) so F^2 is the
                # cosine window; h = -e*w + 2*ln(max(F, tiny))
                t_r = ew_pool.tile([128, HALF], f32, tag="r")
                nc.scalar.activation(t_r[:PUSE], t_w[:PUSE], AF.Sqrt)
                t_f = ew_pool.tile([128, HALF], f32, tag="f")
                nc.scalar.activation(t_f[:PUSE], t_r[:PUSE], AF.Sin,
                                     bias=float(np.pi / 2),
                                     scale=float(-np.pi / (2.0 * rc0)))
                nc.vector.tensor_scalar(t_f[:PUSE], t_f[:PUSE], 1.0e-30,
                                        None, Alu.max)
                t_lf = ew_pool.tile([128, HALF], f32, tag="lf")
                nc.scalar.activation(t_lf[:PUSE], t_f[:PUSE], AF.Ln)
                t_u = ew_pool.tile([128, HALF], f32, tag="u")
                nc.vector.tensor_scalar(t_u[:PUSE], t_w[:PUSE], -float(e0),
                                        None, Alu.mult)
                t_hh = ew_pool.tile([128, HALF], f32, tag="h")
                nc.vector.scalar_tensor_tensor(
                    t_hh[:PUSE], t_lf[:PUSE], 2.0, t_u[:PUSE],
                    op0=Alu.mult, op1=Alu.add)

                # psum: po = atom-in-qt-block, col = qt*128 + (l*4+a)*2 + h
                t_psum = psum_pool.tile([128, HALF], f32, tag="ps")
                for l in range([0, L]["noll" not in ablate]):
                    rs_l = float(rs_vals[l])
                    t_q = k_pool.tile([128, HALF], f32, tag="q")
                    nc.vector.scalar_tensor_tensor(
                        t_q[:PUSE], t_r[:PUSE], 2.0 * float(e0) * rs_l,
                        t_hh[:PUSE], op0=Alu.mult, op1=Alu.add)
                    t_k = k_pool.tile([128, HALF], bf16, tag="k")
                    nc.scalar.activation(t_k[:PUSE], t_q[:PUSE], AF.Exp,
                                         bias=-float(e0) * rs_l * rs_l)
                    for a in range(A):
                        t_p = p_pool.tile([128, HALF], bf16, tag="p")
                        nc.vector.tensor_tensor(
                            t_p[:PUSE], t_k[:PUSE], masks[a][:PUSE],
                            Alu.mult)
                        col = (l * A + a) * 2
                        for qt in range(NQT):
                            nc.tensor.matmul(
                                t_psum[:, qt * 128 + col:qt * 128 + col + 2],
                                t_p[:PUSE, qt * 128:(qt + 1) * 128],
                                t_h[:PUSE, 0:2],
                                start=True, stop=True)

                t_o = out_pool.tile([128, HALF], f32, tag="o")
                if "noll" in ablate:
                    nc.gpsimd.memset(t_o[:], 0.0)
                else:
                    nc.vector.tensor_copy(t_o[:], t_psum[:])

                nc.sync.dma_start(
                    out_d[g * 128:(g + 1) * 128, :], t_o[:])

    nc.compile()
    return nc


def _host_prep(X, Nbrs, Nbrs_Z, atom_types):
    """Type-sort neighbors, pad per atom to SA slots; collect dropped pairs."""
    tid_lut = np.full(256, 255, dtype=np.uint8)
    tid_lut[np.asarray(atom_types, dtype=np.int64)] = np.arange(
        A, dtype=np.uint8)
    tid = tid_lut[Nbrs_Z]                                   # (B,N,M)

    order = np.argsort(tid, axis=-1, kind="stable")
    tid_s = np.take_along_axis(tid, order, axis=-1)         # (B,N,M)
    nbr_s = np.take_along_axis(Nbrs, order, axis=-1)

    typed = tid_s != 255
    slot_idx = np.where(typed[..., :SA], nbr_s[..., :SA], 0).astype(np.int32)
    slot_type = np.where(typed[..., :SA], tid_s[..., :SA], 255).astype(
        np.uint8)

    over = typed[..., SA:]
    drop_b, drop_n, drop_m = np.nonzero(over)
    drop_a = tid_s[drop_b, drop_n, drop_m + SA].astype(np.int64)
    drop_j = nbr_s[drop_b, drop_n, drop_m + SA].astype(np.int64)
    return slot_idx, slot_type, (drop_b, drop_n, drop_a, drop_j)


def _host_correction(out, X, drops, rc, rs, e):
    b, n, a, j = drops
    if len(b) == 0:
        return
    diff = X[b, j].astype(np.float64) - X[b, n].astype(np.float64)
    R = np.sqrt((diff * diff).sum(-1))                      # (D,)
    rc64, rs64, e64 = (np.asarray(v, dtype=np.float64) for v in (rc, rs, e))
    K = np.exp(-e64[None, :] * (R[:, None] - rs64[None, :]) ** 2)
    FC = np.where(R[:, None] <= rc64[None, :],
                  0.5 * (np.cos(np.pi * R[:, None] / rc64[None, :]) + 1.0),
                  0.0)
    contrib = (K * FC)                                      # (D, L)
    la = (np.arange(L)[None, :] * A + a[:, None])           # (D, L)
    flat = out.reshape(L * A, B * N)
    np.add.at(flat, (la.ravel(), np.repeat(b * N + n, L)),
              contrib.astype(np.float32).ravel())


def kernel(X, Nbrs, Nbrs_Z, rc, rs, e, atom_types):
    import ml_dtypes
    from concourse.bass_utils import run_bass_kernel_spmd

    X = np.asarray(X, dtype=np.float32)
    Nbrs = np.asarray(Nbrs, dtype=np.int32)
    Nbrs_Z = np.asarray(Nbrs_Z, dtype=np.int32)
    rc = np.asarray(rc, dtype=np.float32)
    rs = np.asarray(rs, dtype=np.float32)
    e = np.asarray(e, dtype=np.float32)
    atom_types = np.asarray(atom_types, dtype=np.int32)

    assert np.all(rc == rc[0]) and np.all(e == e[0]), \
        "fast path requires uniform rc and e"

    slot_idx, slot_type, drops = _host_prep(X, Nbrs, Nbrs_Z, atom_types)

    import os
    ablate = os.environ.get("KERNEL_ABLATE", "")
    key = (float(rc[0]), float(e[0]), tuple(np.round(rs.astype(float), 9)),
           ablate)
    if key not in _cache:
        _cache[key] = _build_program(float(rc[0]), float(e[0]),
                                     [float(v) for v in rs], ablate)
    nc = _cache[key]

    bf16 = ml_dtypes.bfloat16
    # h-selector for the PE contraction: rows (h*SA+s) -> column h
    hsel_np = np.zeros((128, 2), dtype=bf16)
    for h in range(2):
        hsel_np[h * SA:(h + 1) * SA, h] = 1.0
    # center-broadcast selector: row h -> columns (h*SA+s)
    hsel2_np = np.zeros((128, PUSE), dtype=np.float32)
    for h in range(2):
        hsel2_np[h, h * SA:(h + 1) * SA] = 1.0

    in_maps = []
    for core in range(NCORES):
        bs = core * BPC
        coords = X[bs:bs + BPC].reshape(AT, 3)               # local atoms
        sl = (slot_idx[bs:bs + BPC].astype(np.int64)
              + (np.arange(BPC, dtype=np.int64) * N)[:, None, None]
              ).reshape(AT, SA)
        st = slot_type[bs:bs + BPC].reshape(AT, SA)

        # table rows: tbl32[4k+c] = component c (same for every k)
        tbl = np.zeros((32, NE), dtype=np.float32)
        for k in range(NCORES):
            for c in range(3):
                tbl[4 * k + c, :] = coords[:, c]

        # index tiles: chunk g, Q7 core k covers f-block k.
        # position j = (h*SA+s)*FPC + fl ; atom = g*CH + h*HALF + k*FPC + fl
        idx_np = np.zeros((128, NCHUNK * ICOL), dtype=np.int16)
        hh = np.arange(2)[:, None, None]
        ss = np.arange(SA)[None, :, None]
        fl = np.arange(FPC)[None, None, :]
        jpos = np.arange(2 * SA * FPC)
        for g in range(NCHUNK):
            for k in range(NCORES):
                at = g * CH + hh * HALF + k * FPC + fl       # (2,SA,FPC)
                av = np.broadcast_to(at, (2, SA, FPC))
                sv = np.broadcast_to(ss, (2, SA, FPC))
                vals = sl[av, sv].astype(np.int16).reshape(2 * SA * FPC)
                tile = np.zeros((16, ICOL), dtype=np.int16)
                tile[jpos % 16, jpos // 16] = vals
                idx_np[16 * k:16 * k + 16, g * ICOL:(g + 1) * ICOL] = tile

        # compact type grid: (PUSE, HALF) per chunk, pad slots = 255
        grid_np = np.zeros((128, NCHUNK * HALF), dtype=bf16)
        for g in range(NCHUNK):
            stc = st[g * CH:(g + 1) * CH].reshape(2, HALF, SA)
            grid = stc.transpose(0, 2, 1).reshape(PUSE, HALF)
            grid_np[:PUSE, g * HALF:(g + 1) * HALF] = grid.astype(bf16)

        in_maps.append({"tbl": tbl, "idx": idx_np, "grid": grid_np,
                        "hsel": hsel_np, "hsel2": hsel2_np})

    res = run_bass_kernel_spmd(nc, in_maps, core_ids=list(range(NCORES)))
    global _last_nc, _last_in_maps
    _last_nc, _last_in_maps = nc, in_maps

    out = np.empty((L * A, B, N), dtype=np.float32)
    for core in range(NCORES):
        # raw[g, po, qt, la, h] -> atom = g*CH + h*HALF + qt*128 + po
        raw = res.results[core]["out"].reshape(NCHUNK, 128, NQT, L * A, 2)
        perm = raw.transpose(3, 0, 4, 2, 1)     # (la, g, h, qt, po)
        out[:, core * BPC:(core + 1) * BPC, :] = perm.reshape(
            L * A, BPC, N)

    _host_correction(out, X, drops, rc, rs, e)
    return out


def benchmark(n_pairs=14, klo=2, khi=22):
    """Steady-state per-execution device time of the compiled program.

    Chains k executions of the single jitted bass program (async dispatch
    pipelines them on the device) and takes the median of paired slopes
    (t_khi - t_klo) / (khi - klo), which cancels the per-call dispatch
    overhead of the remote tunnel."""
    import time
    import jax
    import numpy as np
    from jax.sharding import Mesh, PartitionSpec, NamedSharding
    from jax.experimental.shard_map import shard_map
    from concourse import mybir
    from concourse.bass2jax import (_bass_exec_p, install_neuronx_cc_hook,
                                    partition_id_tensor)

    nc, in_maps = _last_nc, _last_in_maps
    install_neuronx_cc_hook()
    partition_name = (nc.partition_id_tensor.name
                      if nc.partition_id_tensor else None)
    in_names, out_names, out_avals, zero_outs = [], [], [], []
    for alloc in nc.m.functions[0].allocations:
        if not isinstance(alloc, mybir.MemoryLocationSet):
            continue
        name = alloc.memorylocations[0].name
        if alloc.kind == "ExternalInput":
            if name != partition_name:
                in_names.append(name)
        elif alloc.kind == "ExternalOutput":
            shape = tuple(alloc.tensor_shape)
            dtype = mybir.dt.np(alloc.dtype)
            out_names.append(name)
            out_avals.append(jax.core.ShapedArray(shape, dtype))
            zero_outs.append(np.zeros(shape, dtype))
    n_params = len(in_names)
    all_in_names = in_names + out_names + (
        [partition_name] if partition_name else [])

    def _body(*args):
        ins = list(args[:n_params])
        outs = list(args[n_params:])
        operands = ins + outs
        if partition_name is not None:
            operands.append(partition_id_tensor())
        outs = list(_bass_exec_p.bind(
            *operands, out_avals=tuple(out_avals),
            in_names=tuple(all_in_names), out_names=tuple(out_names),
            lowering_input_output_aliases=(),
            sim_require_finite=True, sim_require_nnan=True, nc=nc))
        return tuple(outs)

    devices = jax.devices()[:NCORES]
    mesh = Mesh(np.asarray(devices), ("core",))
    fn = jax.jit(shard_map(
        _body, mesh=mesh,
        in_specs=(PartitionSpec("core"),) * (n_params + len(out_names)),
        out_specs=(PartitionSpec("core"),) * len(out_names),
        check_rep=False), keep_unused=True)
    concat_in = [np.concatenate([np.asarray(m[nm]) for m in in_maps], axis=0)
                 for nm in in_names]
    concat_zeros = [np.zeros((NCORES * z.shape[0], *z.shape[1:]), z.dtype)
                    for z in zero_outs]
    sh = NamedSharding(mesh, PartitionSpec("core"))
    ins_dev = [jax.device_put(a, sh) for a in concat_in]
    outs_dev = tuple(jax.device_put(a, sh) for a in concat_zeros)

    def chain(k):
        outs = outs_dev
        for _ in range(k):
            outs = fn(*ins_dev, *outs)
        jax.block_until_ready(outs)

    chain(1)
    slopes = []
    for _ in range(n_pairs):
        t0 = time.perf_counter(); chain(klo); a = time.perf_counter() - t0
        t0 = time.perf_counter(); chain(khi); b = time.perf_counter() - t0
        slopes.append((b - a) / (khi - klo))
    slopes = np.array(slopes)
    return float(np.median(slopes)), float(np.percentile(slopes, 75))


# revision 12
# speedup vs baseline: 56.2659x; 1.1938x over previous
"""AtomicConv radial symmetry function kernel for Trainium2 (8 NeuronCores).

Strategy (v3):
  - Data-parallel over batch: 4 examples per core (8192 atoms per core).
  - Host sorts each atom's 64 neighbors by atom-type and keeps the typed
    ones, padded at the ATOM level to SA=36 slots (excess beyond 36 is
    host-corrected; ~0.04% of atoms). Pad slots point at table index 0 and
    are killed by type masks computed on device from a compact bf16
    type-grid, so the old per-type 4x16 slot padding (64 gather indices
    per atom) shrinks to 36.
  - Device per chunk of 1024 atoms (2 halves x 512 on the free dim, slot
    grid p = h*36+s on partitions):
      gpsimd.ap_gather fetches neighbor x/y/z (per-Q7-core index lists,
      components as table rows 16k+4c) -> one strided SBUF->SBUF DMA per
      component transposes into the slot grid -> chunk centers are
      broadcast across the 36 slot rows with a tiny table-slice DMA + PE
      selector matmul (no replicated center upload) -> DVE computes
      clamped R^2 (subtracting centers straight out of PSUM), ACT computes
      R, the cosine window F and ln F; per radial filter l the Gaussian is
      exp(a_l*R + h + b_l) with h = -e*R^2 + 2 ln F, written in bf16 ->
      DVE multiplies by the 4 bf16 type-mask planes (2x mode) -> PE
      contracts the 72 slot-grid partitions per (l, a) with an h-selector
      into PSUM (atoms on the psum partition dim) -> one drain + one
      store per chunk.
  - DMAs are split between the two HWDGE queues (SP + Activation issue
    engines); the gather (~34 ns per Q7-core index, SBUF read-command
    latency bound) is the bottleneck and everything else hides under it.
"""

import numpy as np

B, N, M = 32, 2048, 64
L, A = 16, 4
NCORES = 8
BPC = B // NCORES          # examples per core
AT = BPC * N               # atoms per core (8192)
SA = 36                    # slots per atom (atom-level padding)
CH = 1024                  # atoms per chunk
HALF = CH // 2             # 512 atoms per half -> free dim
NCHUNK = AT // CH          # 8
PUSE = 2 * SA              # used partitions in the slot grid (72)
NIDX = SA * CH // NCORES   # gathered idxs per Q7 core per chunk (4608)
ICOL = NIDX // 16          # idx cols per chunk (288)
FPC = HALF // NCORES       # atoms (free positions) per Q7 core (64)
NE = AT                    # gather table entries
NQT = HALF // 128          # psum partition blocks per half (4)

_cache = {}


def _build_program(rc0, e0, rs_vals, ablate=""):
    import concourse.bacc as bacc
    import concourse.mybir as mybir
    from concourse.tile import TileContext

    f32 = mybir.dt.float32
    bf16 = mybir.dt.bfloat16
    i16 = mybir.dt.int16
    AF = mybir.ActivationFunctionType
    Alu = mybir.AluOpType

    nc = bacc.Bacc(None, target_bir_lowering=False)

    # register const APs for every activation bias value we use
    bias_vals = {float(np.pi / 2), 1.0e-38}
    for l in range(L):
        bias_vals.add(-float(e0) * float(rs_vals[l]) * float(rs_vals[l]))
    for v in sorted(bias_vals):
        if (f32, v) not in nc.const_aps.aps:
            t = nc.alloc_sbuf_tensor(f"cst-{v!r}", [128, 1], f32)
            nc.gpsimd.memset(t.ap(), v)
            nc.const_aps.aps[(f32, v)] = t.ap()
    nc.all_engine_barrier()

    tbl_d = nc.dram_tensor("tbl", (32, NE), f32, kind="ExternalInput")
    idx_d = nc.dram_tensor("idx", (128, NCHUNK * ICOL), i16,
                           kind="ExternalInput")
    grid_d = nc.dram_tensor("grid", (128, NCHUNK * HALF), bf16,
                            kind="ExternalInput")
    hsel_d = nc.dram_tensor("hsel", (128, 2), bf16, kind="ExternalInput")
    hsel2_d = nc.dram_tensor("hsel2", (128, PUSE), f32, kind="ExternalInput")
    out_d = nc.dram_tensor("out", (NCHUNK * 128, HALF), f32,
                           kind="ExternalOutput")

    rc2 = float(rc0) * float(rc0)

    with TileContext(nc) as tc:
        with (
            tc.tile_pool(name="tab", bufs=1) as tab_pool,
            tc.tile_pool(name="gath", bufs=2) as gath_pool,
            tc.tile_pool(name="idxp", bufs=2) as idx_pool,
            tc.tile_pool(name="grdp", bufs=2) as grd_pool,
            tc.tile_pool(name="ccp", bufs=2) as cc_pool,
            tc.tile_pool(name="mskp", bufs=2) as msk_pool,
            tc.tile_pool(name="comp", bufs=2) as comp_pool,
            tc.tile_pool(name="ew", bufs=2) as ew_pool,
            tc.tile_pool(name="qt", bufs=4) as q_pool,
            tc.tile_pool(name="kt", bufs=2) as k_pool,
            tc.tile_pool(name="pt", bufs=6) as p_pool,
            tc.tile_pool(name="ot", bufs=2) as out_pool,
            tc.tile_pool(name="ps", bufs=2, space="PSUM") as psum_pool,
            tc.tile_pool(name="psc", bufs=2, space="PSUM") as psc_pool,
        ):
            # table rows: partition 4*(4k+c) holds component c for Q7 core k
            t_tab = tab_pool.tile([128, NE], f32)
            nc.sync.dma_start(t_tab[0::4, :], tbl_d[:])
            t_h = tab_pool.tile([128, 2], bf16)
            nc.sync.dma_start(t_h[:], hsel_d[:])
            t_h2 = tab_pool.tile([128, PUSE], f32)
            nc.sync.dma_start(t_h2[:], hsel2_d[:])

            for g in range(NCHUNK):
                t_idx = idx_pool.tile([128, ICOL], i16, tag="idx")
                nc.sync.dma_start(t_idx[:], idx_d[:, g * ICOL:(g + 1) * ICOL])
                t_grid = grd_pool.tile([128, HALF], bf16, tag="grid")
                nc.scalar.dma_start(
                    t_grid[:PUSE], grid_d[:PUSE, g * HALF:(g + 1) * HALF])

                t_g = gath_pool.tile([128, NIDX], f32, tag="g")
                if "nogather" in ablate:
                    nc.gpsimd.memset(t_g[:, 0:4], 0.0)
                else:
                    nc.gpsimd.ap_gather(
                        t_g[:], t_tab[:], t_idx[:],
                        channels=128, num_elems=NE, d=1, num_idxs=NIDX,
                    )

                # chunk centers: tiny table-slice DMA to [2, HALF], then
                # PE broadcast across the 36 slot rows -> psum [PUSE, HALF]
                cenps = []
                for c in range(3):
                    t_cc = cc_pool.tile([128, HALF], f32, tag=f"cc{c}")
                    src = t_tab[4 * c:4 * c + 1,
                                g * CH:(g + 1) * CH].rearrange(
                        "one (h f) -> one h f", h=2)
                    eng = nc.scalar if c != 1 else nc.sync
                    eng.dma_start(t_cc[0:2, :], src)
                    t_pc = psc_pool.tile([128, HALF], f32, tag=f"cps{c}")
                    nc.tensor.matmul(t_pc[0:PUSE, :], t_h2[0:2, 0:PUSE],
                                     t_cc[0:2, :], start=True, stop=True)
                    cenps.append(t_pc)

                # type-mask planes from the compact grid
                masks = []
                for a in range(A):
                    t_m = msk_pool.tile([128, HALF], bf16, tag=f"m{a}")
                    nc.vector.tensor_scalar(t_m[:PUSE], t_grid[:PUSE],
                                            float(a), None, Alu.is_equal)
                    masks.append(t_m)

                # transpose gathered comps into slot grid (p=(h,s), f=atom)
                comps = []
                for c in range(3):
                    t_x = comp_pool.tile([128, HALF], f32, tag=f"comp{c}")
                    if "notrans" in ablate:
                        nc.gpsimd.memset(t_x[:, 0:4], 0.0)
                    else:
                        for k in range(NCORES):
                            row = 16 * k + 4 * c
                            src = t_g[row:row + 1, :].rearrange(
                                "one (p f) -> one p f", p=PUSE)
                            eng = nc.sync if (c + k) % 2 else nc.scalar
                            eng.dma_start(
                                t_x[:PUSE, FPC * k:FPC * k + FPC], src)
                    comps.append(t_x)

                # r2 = sum_c (xj - cx)^2 (centers read from psum), clamped
                t_w = ew_pool.tile([128, HALF], f32, tag="w")
                t_d0 = ew_pool.tile([128, HALF], f32, tag="d0")
                nc.vector.tensor_tensor(t_d0[:PUSE], comps[0][:PUSE],
                                        cenps[0][:PUSE], Alu.subtract)
                nc.vector.tensor_tensor(t_w[:PUSE], t_d0[:PUSE], t_d0[:PUSE],
                                        Alu.mult)
                for c in (1, 2):
                    t_dc = ew_pool.tile([128, HALF], f32, tag="d1")
                    nc.vector.tensor_tensor(t_dc[:PUSE], comps[c][:PUSE],
                                            cenps[c][:PUSE], Alu.subtract)
                    t_sq = ew_pool.tile([128, HALF], f32, tag="sq")
                    nc.vector.tensor_tensor(t_sq[:PUSE], t_dc[:PUSE],
                                            t_dc[:PUSE], Alu.mult)
                    nc.vector.tensor_tensor(t_w[:PUSE], t_w[:PUSE],
                                            t_sq[:PUSE], Alu.add)
                nc.vector.tensor_scalar(t_w[:PUSE], t_w[:PUSE], rc2, None,
                                        Alu.min)

                # R = sqrt(w);  F = sin(pi/2 - pi R/(2rc)) so F^2 is the
                # cosine window; h = -e*w + 2*ln(max(F, tiny))
                t_r = ew_pool.tile([128, HALF], f32, tag="r")
                nc.scalar.activation(t_r[:PUSE], t_w[:PUSE], AF.Sqrt)
                t_f = ew_pool.tile([128, HALF], f32, tag="f")
                nc.scalar.activation(t_f[:PUSE], t_r[:PUSE], AF.Sin,
                                     bias=float(np.pi / 2),
                                     scale=float(-np.pi / (2.0 * rc0)))
                nc.vector.tensor_scalar(t_f[:PUSE], t_f[:PUSE], 1.0e-30,
                                        None, Alu.max)
                t_lf = ew_pool.tile([128, HALF], f32, tag="lf")
                nc.scalar.activation(t_lf[:PUSE], t_f[:PUSE], AF.Ln)
                t_u = ew_pool.tile([128, HALF], f32, tag="u")
                nc.vector.tensor_scalar(t_u[:PUSE], t_w[:PUSE], -float(e0),
                                        None, Alu.mult)
                t_hh = ew_pool.tile([128, HALF], f32, tag="h")
                nc.vector.scalar_tensor_tensor(
                    t_hh[:PUSE], t_lf[:PUSE], 2.0, t_u[:PUSE],
                    op0=Alu.mult, op1=Alu.add)

                # psum row = (l*A+a)*2 + h, f = atom-in-half.
                # Two phases so the DVE queue never waits on ACT results:
                # all (affine, exp) pairs first, then all mask-mults+matmuls.
                t_psum = psum_pool.tile([128, HALF], f32, tag="ps")
                nl = [0, L]["noll" not in ablate]
                ks = []
                for l in range(nl):
                    rs_l = float(rs_vals[l])
                    t_q = q_pool.tile([128, HALF], f32, tag="q")
                    nc.vector.scalar_tensor_tensor(
                        t_q[:PUSE], t_r[:PUSE], 2.0 * float(e0) * rs_l,
                        t_hh[:PUSE], op0=Alu.mult, op1=Alu.add)
                    t_k = k_pool.tile([128, HALF], bf16, tag=f"k{l}")
                    nc.scalar.activation(t_k[:PUSE], t_q[:PUSE], AF.Exp,
                                         bias=-float(e0) * rs_l * rs_l)
                    ks.append(t_k)
                for l in range(nl):
                    for a in range(A):
                        t_p = p_pool.tile([128, HALF], bf16, tag="p")
                        nc.vector.tensor_tensor(
                            t_p[:PUSE], ks[l][:PUSE], masks[a][:PUSE],
                            Alu.mult)
                        col = (l * A + a) * 2
                        for qt in range(NQT):
                            nc.tensor.matmul(
                                t_psum[:, qt * 128 + col:qt * 128 + col + 2],
                                t_p[:PUSE, qt * 128:(qt + 1) * 128],
                                t_h[:PUSE, 0:2],
                                start=True, stop=True)

                t_o = out_pool.tile([128, HALF], f32, tag="o")
                if "noll" in ablate:
                    nc.gpsimd.memset(t_o[:], 0.0)
                else:
                    nc.vector.tensor_copy(t_o[:], t_psum[:])

                nc.sync.dma_start(
                    out_d[g * 128:(g + 1) * 128, :], t_o[:])

    nc.compile()
    return nc


def _host_prep(X, Nbrs, Nbrs_Z, atom_types):
    """Type-sort neighbors, pad per atom to SA slots; collect dropped pairs."""
    tid_lut = np.full(256, 255, dtype=np.uint8)
    tid_lut[np.asarray(atom_types, dtype=np.int64)] = np.arange(
        A, dtype=np.uint8)
    tid = tid_lut[Nbrs_Z]                                   # (B,N,M)

    order = np.argsort(tid, axis=-1, kind="stable")
    tid_s = np.take_along_axis(tid, order, axis=-1)         # (B,N,M)
    nbr_s = np.take_along_axis(Nbrs, order, axis=-1)

    typed = tid_s != 255
    slot_idx = np.where(typed[..., :SA], nbr_s[..., :SA], 0).astype(np.int32)
    slot_type = np.where(typed[..., :SA], tid_s[..., :SA], 255).astype(
        np.uint8)

    over = typed[..., SA:]
    drop_b, drop_n, drop_m = np.nonzero(over)
    drop_a = tid_s[drop_b, drop_n, drop_m + SA].astype(np.int64)
    drop_j = nbr_s[drop_b, drop_n, drop_m + SA].astype(np.int64)
    return slot_idx, slot_type, (drop_b, drop_n, drop_a, drop_j)


def _host_correction(out, X, drops, rc, rs, e):
    b, n, a, j = drops
    if len(b) == 0:
        return
    diff = X[b, j].astype(np.float64) - X[b, n].astype(np.float64)
    R = np.sqrt((diff * diff).sum(-1))                      # (D,)
    rc64, rs64, e64 = (np.asarray(v, dtype=np.float64) for v in (rc, rs, e))
    K = np.exp(-e64[None, :] * (R[:, None] - rs64[None, :]) ** 2)
    FC = np.where(R[:, None] <= rc64[None, :],
                  0.5 * (np.cos(np.pi * R[:, None] / rc64[None, :]) + 1.0),
                  0.0)
    contrib = (K * FC)                                      # (D, L)
    la = (np.arange(L)[None, :] * A + a[:, None])           # (D, L)
    flat = out.reshape(L * A, B * N)
    np.add.at(flat, (la.ravel(), np.repeat(b * N + n, L)),
              contrib.astype(np.float32).ravel())


def kernel(X, Nbrs, Nbrs_Z, rc, rs, e, atom_types):
    import ml_dtypes
    from concourse.bass_utils import run_bass_kernel_spmd

    X = np.asarray(X, dtype=np.float32)
    Nbrs = np.asarray(Nbrs, dtype=np.int32)
    Nbrs_Z = np.asarray(Nbrs_Z, dtype=np.int32)
    rc = np.asarray(rc, dtype=np.float32)
    rs = np.asarray(rs, dtype=np.float32)
    e = np.asarray(e, dtype=np.float32)
    atom_types = np.asarray(atom_types, dtype=np.int32)

    assert np.all(rc == rc[0]) and np.all(e == e[0]), \
        "fast path requires uniform rc and e"

    slot_idx, slot_type, drops = _host_prep(X, Nbrs, Nbrs_Z, atom_types)

    import os
    ablate = os.environ.get("KERNEL_ABLATE", "")
    key = (float(rc[0]), float(e[0]), tuple(np.round(rs.astype(float), 9)),
           ablate)
    if key not in _cache:
        _cache[key] = _build_program(float(rc[0]), float(e[0]),
                                     [float(v) for v in rs], ablate)
    nc = _cache[key]

    bf16 = ml_dtypes.bfloat16
    # h-selector for the PE contraction: rows (h*SA+s) -> column h
    hsel_np = np.zeros((128, 2), dtype=bf16)
    for h in range(2):
        hsel_np[h * SA:(h + 1) * SA, h] = 1.0
    # center-broadcast selector: row h -> columns (h*SA+s)
    hsel2_np = np.zeros((128, PUSE), dtype=np.float32)
    for h in range(2):
        hsel2_np[h, h * SA:(h + 1) * SA] = 1.0

    in_maps = []
    for core in range(NCORES):
        bs = core * BPC
        coords = X[bs:bs + BPC].reshape(AT, 3)               # local atoms
        sl = (slot_idx[bs:bs + BPC].astype(np.int64)
              + (np.arange(BPC, dtype=np.int64) * N)[:, None, None]
              ).reshape(AT, SA)
        st = slot_type[bs:bs + BPC].reshape(AT, SA)

        # table rows: tbl32[4k+c] = component c (same for every k)
        tbl = np.zeros((32, NE), dtype=np.float32)
        for k in range(NCORES):
            for c in range(3):
                tbl[4 * k + c, :] = coords[:, c]

        # index tiles: chunk g, Q7 core k covers f-block k.
        # position j = (h*SA+s)*FPC + fl ; atom = g*CH + h*HALF + k*FPC + fl
        idx_np = np.zeros((128, NCHUNK * ICOL), dtype=np.int16)
        hh = np.arange(2)[:, None, None]
        ss = np.arange(SA)[None, :, None]
        fl = np.arange(FPC)[None, None, :]
        jpos = np.arange(2 * SA * FPC)
        for g in range(NCHUNK):
            for k in range(NCORES):
                at = g * CH + hh * HALF + k * FPC + fl       # (2,SA,FPC)
                av = np.broadcast_to(at, (2, SA, FPC))
                sv = np.broadcast_to(ss, (2, SA, FPC))
                vals = sl[av, sv].astype(np.int16).reshape(2 * SA * FPC)
                tile = np.zeros((16, ICOL), dtype=np.int16)
                tile[jpos % 16, jpos // 16] = vals
                idx_np[16 * k:16 * k + 16, g * ICOL:(g + 1) * ICOL] = tile

        # compact type grid: (PUSE, HALF) per chunk, pad slots = 255
        grid_np = np.zeros((128, NCHUNK * HALF), dtype=bf16)
        for g in range(NCHUNK):
            stc = st[g * CH:(g + 1) * CH].reshape(2, HALF, SA)
            grid = stc.transpose(0, 2, 1).reshape(PUSE, HALF)
            grid_np[:PUSE, g * HALF:(g + 1) * HALF] = grid.astype(bf16)

        in_maps.append({"tbl": tbl, "idx": idx_np, "grid": grid_np,
                        "hsel": hsel_np, "hsel2": hsel2_np})

    res = run_bass_kernel_spmd(nc, in_maps, core_ids=list(range(NCORES)))
    global _last_nc, _last_in_maps
    _last_nc, _last_in_maps = nc, in_maps

    out = np.empty((L * A, B, N), dtype=np.float32)
    for core in range(NCORES):
        # raw[g, po, qt, la, h] -> atom = g*CH + h*HALF + qt*128 + po
        raw = res.results[core]["out"].reshape(NCHUNK, 128, NQT, L * A, 2)
        perm = raw.transpose(3, 0, 4, 2, 1)     # (la, g, h, qt, po)
        out[:, core * BPC:(core + 1) * BPC, :] = perm.reshape(
            L * A, BPC, N)

    _host_correction(out, X, drops, rc, rs, e)
    return out


def benchmark(n_pairs=14, klo=2, khi=22):
    """Steady-state per-execution device time of the compiled program.

    Chains k executions of the single jitted bass program (async dispatch
    pipelines them on the device) and takes the median of paired slopes
    (t_khi - t_klo) / (khi - klo), which cancels the per-call dispatch
    overhead of the remote tunnel."""
    import time
    import jax
    import numpy as np
    from jax.sharding import Mesh, PartitionSpec, NamedSharding
    from jax.experimental.shard_map import shard_map
    from concourse import mybir
    from concourse.bass2jax import (_bass_exec_p, install_neuronx_cc_hook,
                                    partition_id_tensor)

    nc, in_maps = _last_nc, _last_in_maps
    install_neuronx_cc_hook()
    partition_name = (nc.partition_id_tensor.name
                      if nc.partition_id_tensor else None)
    in_names, out_names, out_avals, zero_outs = [], [], [], []
    for alloc in nc.m.functions[0].allocations:
        if not isinstance(alloc, mybir.MemoryLocationSet):
            continue
        name = alloc.memorylocations[0].name
        if alloc.kind == "ExternalInput":
            if name != partition_name:
                in_names.append(name)
        elif alloc.kind == "ExternalOutput":
            shape = tuple(alloc.tensor_shape)
            dtype = mybir.dt.np(alloc.dtype)
            out_names.append(name)
            out_avals.append(jax.core.ShapedArray(shape, dtype))
            zero_outs.append(np.zeros(shape, dtype))
    n_params = len(in_names)
    all_in_names = in_names + out_names + (
        [partition_name] if partition_name else [])

    def _body(*args):
        ins = list(args[:n_params])
        outs = list(args[n_params:])
        operands = ins + outs
        if partition_name is not None:
            operands.append(partition_id_tensor())
        outs = list(_bass_exec_p.bind(
            *operands, out_avals=tuple(out_avals),
            in_names=tuple(all_in_names), out_names=tuple(out_names),
            lowering_input_output_aliases=(),
            sim_require_finite=True, sim_require_nnan=True, nc=nc))
        return tuple(outs)

    devices = jax.devices()[:NCORES]
    mesh = Mesh(np.asarray(devices), ("core",))
    fn = jax.jit(shard_map(
        _body, mesh=mesh,
        in_specs=(PartitionSpec("core"),) * (n_params + len(out_names)),
        out_specs=(PartitionSpec("core"),) * len(out_names),
        check_rep=False), keep_unused=True)
    concat_in = [np.concatenate([np.asarray(m[nm]) for m in in_maps], axis=0)
                 for nm in in_names]
    concat_zeros = [np.zeros((NCORES * z.shape[0], *z.shape[1:]), z.dtype)
                    for z in zero_outs]
    sh = NamedSharding(mesh, PartitionSpec("core"))
    ins_dev = [jax.device_put(a, sh) for a in concat_in]
    outs_dev = tuple(jax.device_put(a, sh) for a in concat_zeros)

    def chain(k):
        outs = outs_dev
        for _ in range(k):
            outs = fn(*ins_dev, *outs)
        jax.block_until_ready(outs)

    chain(1)
    slopes = []
    for _ in range(n_pairs):
        t0 = time.perf_counter(); chain(klo); a = time.perf_counter() - t0
        t0 = time.perf_counter(); chain(khi); b = time.perf_counter() - t0
        slopes.append((b - a) / (khi - klo))
    slopes = np.array(slopes)
    return float(np.median(slopes)), float(np.percentile(slopes, 75))
